# revision 23
# baseline (speedup 1.0000x reference)
"""Distributed embedding lookup (bag gather + masked mean) on 8 Trainium2 cores.

Data-parallel over the batch; each core keeps a full table replica in HBM and
handles 512 of 4096 batch rows (13312 slots, partition-tile layout).

The gather is Pool-engine bound (~1.34us per indirect-DMA instruction, 128
rows each), so the row count per instruction column matters. Host-side index
routing compacts each slot's valid keys first:
  - static part: the first C=5 valid keys per slot -> 5*104 gather columns
    (missing keys point at an appended zero row).
  - overflow part (6th..10th valid key, ~16k rows/core): packed densely
    across partitions into per-occurrence column blocks, fetched by the same
    indirect gather, then recombined by slot id with SBUF-dest dma_scatter_add
    (CCE add, parity-split accumulators). Each call spans one occurrence
    block, so indices within a call are slot-unique (the CCE add races on
    intra-call duplicates). The accumulators are folded into the static tree
    before the 1/max(count,1) scale.
"""

import numpy as np

# Problem constants (hardcoded per harness contract).
B, S, N, E, V = 4096, 26, 10, 64, 1_000_000
NCORES = 8
BL = B // NCORES              # 512 batch rows per core
SL = BL * S                   # 13312 slots per core
P = 128
NT = SL // P                  # 104 tiles of 128 slots
C = 5                         # statically gathered keys per slot
GT = 8                        # tiles per gather super-tile
NSUP = NT // GT               # 13
VPAD = V + 8                  # zero sentinel row at index V
DUMP_SLOT = 13440             # tile 105 (odd parity, group 52): trash row
NGRP = NT // 2 + 1            # 53 free-dim groups per parity accumulator

_STATE = {}


def _build_nc(ocols):
    """ocols: tuple of per-occurrence overflow column counts."""
    import concourse.bass as bass
    import concourse.bacc as bacc
    import concourse.mybir as mybir
    import concourse.tile as tile

    f32, i32, i16 = mybir.dt.float32, mybir.dt.int32, mybir.dt.int16
    OC = sum(ocols)

    nc = bacc.Bacc("TRN2", target_bir_lowering=False, debug=False,
                   num_devices=NCORES)
    skeys_t = nc.declare_dram_parameter("skeys_t", [P, NT * C], i32,
                                        isOutput=False)
    okeys_t = nc.declare_dram_parameter("okeys_t", [P, max(OC, 1)], i32,
                                        isOutput=False)
    osid_t = nc.declare_dram_parameter("osid_t", [P, max(OC, 1) * 8], i16,
                                       isOutput=False)
    mask_t = nc.declare_dram_parameter("mask_t", [P, NT * N], i32,
                                       isOutput=False)
    table_t = nc.declare_dram_parameter("table_t", [VPAD, E], f32,
                                        isOutput=False)
    out_t = nc.declare_dram_parameter("out_t", [P, NT * E], f32,
                                      isOutput=True)

    with tile.TileContext(nc) as tc:
        with (
            tc.tile_pool(name="persist", bufs=1) as persist,
            tc.tile_pool(name="gather", bufs=8) as gpool,
            tc.tile_pool(name="tmp", bufs=8) as tpool,
            tc.tile_pool(name="outp", bufs=4) as opool,
        ):
            skeys_sb = persist.tile([P, NT * C], i32)
            okeys_sb = persist.tile([P, max(OC, 1)], i32)
            osid_sb = persist.tile([P, max(OC, 1) * 8], i16)
            mask_sb = persist.tile([P, NT * N], i32)
            counts_i = persist.tile([P, NT], i32)
            counts_f = persist.tile([P, NT], f32)
            recip = persist.tile([P, NT], f32)
            acc_ev = persist.tile([P, NGRP * E], f32)
            acc_od = persist.tile([P, NGRP * E], f32)
            ogt = persist.tile([P, max(OC, 1) * E], f32)

            nc.sync.dma_start(out=skeys_sb[:], in_=skeys_t[:])
            nc.sync.dma_start(out=okeys_sb[:], in_=okeys_t[:])
            nc.sync.dma_start(out=osid_sb[:], in_=osid_t[:])
            nc.sync.dma_start(out=mask_sb[:], in_=mask_t[:])

            nc.vector.memset(acc_ev[:], 0.0)
            nc.vector.memset(acc_od[:], 0.0)

            with nc.allow_low_precision(reason="int32 sum of 10 0/1 values"):
                nc.vector.tensor_reduce(
                    out=counts_i[:],
                    in_=mask_sb[:].rearrange("p (t n) -> p t n", n=N),
                    axis=mybir.AxisListType.X,
                    op=mybir.AluOpType.add,
                )
            nc.vector.tensor_copy(out=counts_f[:], in_=counts_i[:])
            nc.vector.tensor_scalar_max(out=counts_f[:], in0=counts_f[:],
                                        scalar1=1.0)
            nc.vector.reciprocal(out=recip[:], in_=counts_f[:])

            # overflow: gather columns, then slot-unique scatter_adds per
            # occurrence block into the parity-split accumulators.
            for c in range(OC):
                nc.gpsimd.indirect_dma_start(
                    out=ogt[:, c * E:(c + 1) * E],
                    out_offset=None,
                    in_=table_t[:],
                    in_offset=bass.IndirectOffsetOnAxis(
                        ap=okeys_sb[:, c:c + 1], axis=0),
                )
            # scatter sub-call schedule: <=4 columns (512 idxs) keeps
            # single-packet mode within the HW descriptor-ring limit. The
            # calls WAW-serialize on the accumulators, so they are interleaved
            # into the static gather stream below to hide each drain wait.
            scat = []
            off = 0
            for cj in ocols:
                for o2 in range(off, off + cj, 2):
                    scat.append((o2, min(2, off + cj - o2)))
                off += cj

            def emit_scatter(o2, cs):
                nc.gpsimd.dma_scatter_add(
                    out_ap=acc_ev[:],
                    out_ap_other=acc_od[:],
                    in_ap=ogt[:, o2 * E:(o2 + cs) * E]
                    .rearrange("p (c e) -> p c e", e=E),
                    idxs_ap=osid_sb[:, o2 * 8:(o2 + cs) * 8],
                    num_idxs=cs * P,
                    num_idxs_reg=cs * P,
                    elem_size=E,
                    sbuf_tokens_per_rank=P,
                    parity_reg=0,
                )

            # static part: C columns per tile, tree reduce + acc fold +
            # scale. Scatters are spread over the first HD supers (one per
            # ~7 gather columns) to hide their WAW drain waits behind gather
            # work; those supers stage their tree sums and fold + store only
            # after the last scatter (so the accumulators are complete).
            HD = NSUP // 2
            stage = persist.tile([P, HD * GT * E], f32)
            si = 0

            def tree(sl, out_ap):
                t128 = tpool.tile([P, 2 * E], f32)
                nc.vector.tensor_add(out=t128[:], in0=sl[:, 0:2 * E],
                                     in1=sl[:, 2 * E:4 * E])
                nc.vector.tensor_add(out=out_ap, in0=t128[:, 0:E],
                                     in1=t128[:, E:2 * E])
                nc.vector.tensor_add(out=out_ap, in0=out_ap,
                                     in1=sl[:, 4 * E:5 * E])

            def fold(tt, src_ap, osup, i):
                acc = acc_ev if tt % 2 == 0 else acc_od
                aslice = acc[:, (tt // 2) * E:(tt // 2 + 1) * E]
                t64 = tpool.tile([P, E], f32)
                nc.vector.tensor_add(out=t64[:], in0=src_ap, in1=aslice)
                nc.vector.tensor_scalar_mul(
                    out=osup[:, i * E:(i + 1) * E], in0=t64[:],
                    scalar1=recip[:, tt:tt + 1])

            for g in range(NSUP):
                gt = gpool.tile([P, GT * C * E], f32)
                for j in range(GT * C):
                    nc.gpsimd.indirect_dma_start(
                        out=gt[:, j * E:(j + 1) * E],
                        out_offset=None,
                        in_=table_t[:],
                        in_offset=bass.IndirectOffsetOnAxis(
                            ap=skeys_sb[:, g * GT * C + j:g * GT * C + j + 1],
                            axis=0),
                    )
                    if j % 3 == 2 and g < HD and si < len(scat):
                        emit_scatter(*scat[si])
                        si += 1
                if g < HD:
                    # stage tree sums; acc not complete yet
                    for i in range(GT):
                        tt = g * GT + i
                        tree(gt[:, i * C * E:(i + 1) * C * E],
                             stage[:, tt * E:(tt + 1) * E])
                    continue
                if g == HD:
                    while si < len(scat):   # any leftover scatters
                        emit_scatter(*scat[si])
                        si += 1
                    # folds for the staged supers; Pool streams ahead
                    for g2 in range(HD):
                        osup2 = opool.tile([P, GT * E], f32, tag="osup")
                        for i in range(GT):
                            tt = g2 * GT + i
                            fold(tt, stage[:, tt * E:(tt + 1) * E], osup2, i)
                        nc.sync.dma_start(
                            out=out_t[:, g2 * GT * E:(g2 + 1) * GT * E],
                            in_=osup2[:])
                osup = opool.tile([P, GT * E], f32, tag="osup")
                for i in range(GT):
                    tt = g * GT + i
                    sl = gt[:, i * C * E:(i + 1) * C * E]
                    t64b = tpool.tile([P, E], f32)
                    tree(sl, t64b[:])
                    fold(tt, t64b[:], osup, i)
                nc.sync.dma_start(out=out_t[:, g * GT * E:(g + 1) * GT * E],
                                  in_=osup[:])
    nc.compile()
    return nc


def _make_runner(nc):
    import jax
    import concourse.mybir as mybir
    from concourse import bass2jax
    from jax.sharding import Mesh, PartitionSpec
    from jax.experimental.shard_map import shard_map

    bass2jax.install_neuronx_cc_hook()

    in_names, out_names, out_avals, zero_shapes = [], [], [], []
    partition_name = (nc.partition_id_tensor.name
                      if nc.partition_id_tensor else None)
    for alloc in nc.m.functions[0].allocations:
        if not isinstance(alloc, mybir.MemoryLocationSet):
            continue
        name = alloc.memorylocations[0].name
        if alloc.kind == "ExternalInput":
            if name != partition_name:
                in_names.append(name)
        elif alloc.kind == "ExternalOutput":
            out_names.append(name)
            shape = tuple(alloc.tensor_shape)
            dtype = mybir.dt.np(alloc.dtype)
            out_avals.append(jax.core.ShapedArray(shape, dtype))
            zero_shapes.append((shape, dtype))
    n_params = len(in_names)
    n_outs = len(out_avals)
    all_in_names = list(in_names) + list(out_names)
    if partition_name is not None:
        all_in_names.append(partition_name)
    donate = tuple(range(n_params, n_params + n_outs))

    def _body(*args):
        operands = list(args)
        if partition_name is not None:
            operands.append(bass2jax.partition_id_tensor())
        outs = bass2jax._bass_exec_p.bind(
            *operands,
            out_avals=tuple(out_avals),
            in_names=tuple(all_in_names),
            out_names=tuple(out_names),
            lowering_input_output_aliases=(),
            sim_require_finite=True,
            sim_require_nnan=True,
            nc=nc,
        )
        return tuple(outs)

    devices = jax.devices()[:NCORES]
    mesh = Mesh(np.asarray(devices), ("core",))
    specs = [PartitionSpec() if name == "table_t" else PartitionSpec("core")
             for name in in_names]
    in_specs = tuple(specs) + (PartitionSpec("core"),) * n_outs
    out_specs = (PartitionSpec("core"),) * len(out_names)
    fn = jax.jit(
        shard_map(_body, mesh=mesh, in_specs=in_specs, out_specs=out_specs,
                  check_rep=False),
        donate_argnums=donate, keep_unused=True,
    )
    return fn, mesh, in_names, out_names, zero_shapes


def _percore_sorted(keys, mask, c):
    """Per-slot valid-first key ordering for core c."""
    k = np.asarray(keys[c * BL:(c + 1) * BL]).reshape(SL, N)
    m = np.asarray(mask[c * BL:(c + 1) * BL]).reshape(SL, N) != 0
    order = np.argsort(~m, axis=1, kind="stable")
    ksort = np.take_along_axis(k, order, axis=1).astype(np.int64)
    vcnt = m.sum(axis=1)
    return ksort, vcnt, m


def needed_ocols(keys, mask):
    """Per-occurrence overflow column counts (max over cores)."""
    mx = [0] * (N - C)
    for c in range(NCORES):
        _, vcnt, _ = _percore_sorted(keys, mask, c)
        for j in range(N - C):
            cnt = int((vcnt > C + j).sum())
            mx[j] = max(mx[j], (cnt + P - 1) // P)
    while mx and mx[-1] == 0:
        mx.pop()
    return tuple(mx)


def marshal_inputs(keys, mask, ocols):
    OC = sum(ocols)
    ocw = max(OC, 1)
    skeys_g = np.empty((NCORES * P, NT * C), np.int32)
    okeys_g = np.full((NCORES * P, ocw), V, np.int32)
    osid_g = np.full((NCORES * P, ocw * 8), DUMP_SLOT, np.int16)
    mask_g = np.empty((NCORES * P, NT * N), np.int32)
    ooff = np.concatenate(([0], np.cumsum(ocols))).astype(int)
    for c in range(NCORES):
        ksort, vcnt, m = _percore_sorted(keys, mask, c)
        static = ksort[:, :C].copy()
        static[np.arange(C)[None, :] >= vcnt[:, None]] = V
        skeys_g[c * P:(c + 1) * P] = (
            static.reshape(NT, P, C).transpose(1, 0, 2)
            .reshape(P, NT * C).astype(np.int32))
        mask_g[c * P:(c + 1) * P] = (
            m.reshape(NT, P, N).transpose(1, 0, 2)
            .reshape(P, NT * N).astype(np.int32))
        for j in range(len(ocols)):
            sel = np.flatnonzero(vcnt > C + j)       # slots with occurrence j
            kj = ksort[sel, C + j]
            cj = ocols[j]
            if len(sel) > cj * P:
                raise OverflowError(f"occurrence {j}: {len(sel)} > {cj * P}")
            kflat = np.full(cj * P, V, np.int64)
            sflat = np.full(cj * P, DUMP_SLOT, np.int64)
            kflat[:len(sel)] = kj
            sflat[:len(sel)] = sel
            # source cell for stream pos i: partition i%128, col off + i//128
            okeys_g[c * P:(c + 1) * P, ooff[j]:ooff[j] + cj] = (
                kflat.reshape(cj, P).T.astype(np.int32))
            # idx wrapped layout per call: pos i -> [i%16, i//16], replicated
            w = sflat.reshape(cj * 8, 16).T.astype(np.int16)   # [16, cj*8]
            osid_g[c * P:(c + 1) * P, ooff[j] * 8:(ooff[j] + cj) * 8] = (
                np.tile(w, (8, 1)))
    return {"skeys_t": skeys_g, "okeys_t": okeys_g, "osid_t": osid_g,
            "mask_t": mask_g}


def pad_table(table):
    t = np.zeros((VPAD, E), np.float32)
    t[:V] = table
    return t


def unmarshal_output(out_g):
    out = np.empty((B, S, E), np.float32)
    for c in range(NCORES):
        oc = np.asarray(out_g[c * P:(c + 1) * P])
        out[c * BL:(c + 1) * BL] = (
            oc.reshape(P, NT, E).transpose(1, 0, 2).reshape(BL, S, E))
    return out


def _get_state(ocols):
    if _STATE.get("ocols") != ocols:
        nc = _build_nc(ocols)
        fn, mesh, in_names, out_names, zero_shapes = _make_runner(nc)
        _STATE.update(ocols=ocols, nc=nc, fn=fn, mesh=mesh,
                      in_names=in_names, out_names=out_names,
                      zero_shapes=zero_shapes, table_key=None)
    return _STATE


def kernel(keys, mask, table):
    import jax
    from jax.sharding import NamedSharding, PartitionSpec

    ocols = needed_ocols(keys, mask)
    st = _get_state(ocols)
    ins = marshal_inputs(keys, mask, ocols)

    tkey = id(table)
    if st.get("table_key") != tkey:
        st["table_dev"] = jax.device_put(
            pad_table(np.asarray(table, dtype=np.float32)),
            NamedSharding(st["mesh"], PartitionSpec()))
        st["table_key"] = tkey
    ins["table_t"] = st["table_dev"]

    args = [ins[name] for name in st["in_names"]]
    zshape, zdtype = st["zero_shapes"][0]
    zeros_out = np.zeros((NCORES * zshape[0], *zshape[1:]), zdtype)
    outs = st["fn"](*args, zeros_out)
    out_g = np.asarray(jax.block_until_ready(outs[0]))
    return unmarshal_output(out_g)


# revision 25
# speedup vs baseline: 1.0484x; 1.0484x over previous
"""Distributed embedding lookup (bag gather + masked mean) on 8 Trainium2 cores.

Data-parallel over the batch; each core keeps a full table replica in HBM and
handles 512 of 4096 batch rows (13312 slots, partition-tile layout).

The gather is Pool-engine bound (~1.34us per indirect-DMA instruction, 128
rows each), so the row count per instruction column matters. Host-side index
routing compacts each slot's valid keys first:
  - static part: the first C=5 valid keys per slot -> 5*104 gather columns
    (missing keys point at an appended zero row).
  - overflow part (6th..10th valid key, ~16k rows/core): packed densely
    across partitions into per-occurrence column blocks, fetched by the same
    indirect gather, then recombined by slot id with SBUF-dest dma_scatter_add
    (CCE add, parity-split accumulators). Each call spans one occurrence
    block, so indices within a call are slot-unique (the CCE add races on
    intra-call duplicates). The accumulators are folded into the static tree
    before the 1/max(count,1) scale.
"""

import numpy as np

# Problem constants (hardcoded per harness contract).
B, S, N, E, V = 4096, 26, 10, 64, 1_000_000
NCORES = 8
BL = B // NCORES              # 512 batch rows per core
SL = BL * S                   # 13312 slots per core
P = 128
NT = SL // P                  # 104 tiles of 128 slots
C = 5                         # statically gathered keys per slot
GT = 8                        # tiles per gather super-tile
NSUP = NT // GT               # 13
VPAD = V + 8                  # zero sentinel row at index V
DUMP_SLOT = 13440             # tile 105 (odd parity, group 52): trash row
NGRP = NT // 2 + 1            # 53 free-dim groups per parity accumulator

_STATE = {}


def _build_nc(ocols):
    """ocols: tuple of per-occurrence overflow column counts."""
    import concourse.bass as bass
    import concourse.bacc as bacc
    import concourse.mybir as mybir
    import concourse.tile as tile

    f32, i32, i16 = mybir.dt.float32, mybir.dt.int32, mybir.dt.int16
    OC = sum(ocols)

    nc = bacc.Bacc("TRN2", target_bir_lowering=False, debug=False,
                   num_devices=NCORES, num_swdge_queues=2)
    skeys_t = nc.declare_dram_parameter("skeys_t", [P, NT * C], i32,
                                        isOutput=False)
    okeys_t = nc.declare_dram_parameter("okeys_t", [P, max(OC, 1)], i32,
                                        isOutput=False)
    osid_t = nc.declare_dram_parameter("osid_t", [P, max(OC, 1) * 8], i16,
                                       isOutput=False)
    mask_t = nc.declare_dram_parameter("mask_t", [P, NT * N], i32,
                                       isOutput=False)
    table_t = nc.declare_dram_parameter("table_t", [VPAD, E], f32,
                                        isOutput=False)
    out_t = nc.declare_dram_parameter("out_t", [P, NT * E], f32,
                                      isOutput=True)

    with tile.TileContext(nc) as tc:
        with (
            tc.tile_pool(name="persist", bufs=1) as persist,
            tc.tile_pool(name="gather", bufs=8) as gpool,
            tc.tile_pool(name="tmp", bufs=8) as tpool,
            tc.tile_pool(name="outp", bufs=4) as opool,
        ):
            skeys_sb = persist.tile([P, NT * C], i32)
            okeys_sb = persist.tile([P, max(OC, 1)], i32)
            osid_sb = persist.tile([P, max(OC, 1) * 8], i16)
            mask_sb = persist.tile([P, NT * N], i32)
            counts_i = persist.tile([P, NT], i32)
            counts_f = persist.tile([P, NT], f32)
            recip = persist.tile([P, NT], f32)
            acc_ev = persist.tile([P, NGRP * E], f32)
            acc_od = persist.tile([P, NGRP * E], f32)
            ogt = persist.tile([P, max(OC, 1) * E], f32)

            nc.sync.dma_start(out=skeys_sb[:], in_=skeys_t[:])
            nc.sync.dma_start(out=okeys_sb[:], in_=okeys_t[:])
            nc.sync.dma_start(out=osid_sb[:], in_=osid_t[:])
            nc.sync.dma_start(out=mask_sb[:], in_=mask_t[:])

            nc.vector.memset(acc_ev[:], 0.0)
            nc.vector.memset(acc_od[:], 0.0)

            with nc.allow_low_precision(reason="int32 sum of 10 0/1 values"):
                nc.vector.tensor_reduce(
                    out=counts_i[:],
                    in_=mask_sb[:].rearrange("p (t n) -> p t n", n=N),
                    axis=mybir.AxisListType.X,
                    op=mybir.AluOpType.add,
                )
            nc.vector.tensor_copy(out=counts_f[:], in_=counts_i[:])
            nc.vector.tensor_scalar_max(out=counts_f[:], in0=counts_f[:],
                                        scalar1=1.0)
            nc.vector.reciprocal(out=recip[:], in_=counts_f[:])

            # overflow: gather columns, then slot-unique scatter_adds per
            # occurrence block into the parity-split accumulators.
            for c in range(OC):
                nc.gpsimd.indirect_dma_start(
                    out=ogt[:, c * E:(c + 1) * E],
                    out_offset=None,
                    in_=table_t[:],
                    in_offset=bass.IndirectOffsetOnAxis(
                        ap=okeys_sb[:, c:c + 1], axis=0),
                )
            # scatter sub-call schedule: <=4 columns (512 idxs) keeps
            # single-packet mode within the HW descriptor-ring limit. The
            # calls WAW-serialize on the accumulators, so they are interleaved
            # into the static gather stream below to hide each drain wait.
            scat = []
            off = 0
            for cj in ocols:
                for o2 in range(off, off + cj, 8):
                    scat.append((o2, min(8, off + cj - o2)))
                off += cj

            def emit_scatter(o2, cs):
                nc.gpsimd.dma_scatter_add(
                    out_ap=acc_ev[:],
                    out_ap_other=acc_od[:],
                    in_ap=ogt[:, o2 * E:(o2 + cs) * E]
                    .rearrange("p (c e) -> p c e", e=E),
                    idxs_ap=osid_sb[:, o2 * 8:(o2 + cs) * 8],
                    num_idxs=cs * P,
                    num_idxs_reg=cs * P,
                    elem_size=E,
                    sbuf_tokens_per_rank=P,
                    parity_reg=0,
                    queue_num=1,
                )

            # static part: C columns per tile, tree reduce + acc fold +
            # scale. Scatters are spread over the first HD supers (one per
            # ~7 gather columns) to hide their WAW drain waits behind gather
            # work; those supers stage their tree sums and fold + store only
            # after the last scatter (so the accumulators are complete).
            HD = NSUP // 2
            stage = persist.tile([P, HD * GT * E], f32)
            si = 0

            def tree(sl, out_ap):
                t128 = tpool.tile([P, 2 * E], f32)
                nc.vector.tensor_add(out=t128[:], in0=sl[:, 0:2 * E],
                                     in1=sl[:, 2 * E:4 * E])
                nc.vector.tensor_add(out=out_ap, in0=t128[:, 0:E],
                                     in1=t128[:, E:2 * E])
                nc.vector.tensor_add(out=out_ap, in0=out_ap,
                                     in1=sl[:, 4 * E:5 * E])

            def fold(tt, src_ap, osup, i):
                acc = acc_ev if tt % 2 == 0 else acc_od
                aslice = acc[:, (tt // 2) * E:(tt // 2 + 1) * E]
                t64 = tpool.tile([P, E], f32)
                nc.vector.tensor_add(out=t64[:], in0=src_ap, in1=aslice)
                nc.vector.tensor_scalar_mul(
                    out=osup[:, i * E:(i + 1) * E], in0=t64[:],
                    scalar1=recip[:, tt:tt + 1])

            for g in range(NSUP):
                gt = gpool.tile([P, GT * C * E], f32)
                for j in range(GT * C):
                    nc.gpsimd.indirect_dma_start(
                        out=gt[:, j * E:(j + 1) * E],
                        out_offset=None,
                        in_=table_t[:],
                        in_offset=bass.IndirectOffsetOnAxis(
                            ap=skeys_sb[:, g * GT * C + j:g * GT * C + j + 1],
                            axis=0),
                    )
                    if j % 7 == 6 and g < HD and si < len(scat):
                        emit_scatter(*scat[si])
                        si += 1
                if g < HD:
                    # stage tree sums; acc not complete yet
                    for i in range(GT):
                        tt = g * GT + i
                        tree(gt[:, i * C * E:(i + 1) * C * E],
                             stage[:, tt * E:(tt + 1) * E])
                    continue
                if g == HD:
                    while si < len(scat):   # any leftover scatters
                        emit_scatter(*scat[si])
                        si += 1
                    # folds for the staged supers; Pool streams ahead
                    for g2 in range(HD):
                        osup2 = opool.tile([P, GT * E], f32, tag="osup")
                        for i in range(GT):
                            tt = g2 * GT + i
                            fold(tt, stage[:, tt * E:(tt + 1) * E], osup2, i)
                        nc.sync.dma_start(
                            out=out_t[:, g2 * GT * E:(g2 + 1) * GT * E],
                            in_=osup2[:])
                osup = opool.tile([P, GT * E], f32, tag="osup")
                for i in range(GT):
                    tt = g * GT + i
                    sl = gt[:, i * C * E:(i + 1) * C * E]
                    t64b = tpool.tile([P, E], f32)
                    tree(sl, t64b[:])
                    fold(tt, t64b[:], osup, i)
                nc.sync.dma_start(out=out_t[:, g * GT * E:(g + 1) * GT * E],
                                  in_=osup[:])
    nc.compile()
    return nc


def _make_runner(nc):
    import jax
    import concourse.mybir as mybir
    from concourse import bass2jax
    from jax.sharding import Mesh, PartitionSpec
    from jax.experimental.shard_map import shard_map

    bass2jax.install_neuronx_cc_hook()

    in_names, out_names, out_avals, zero_shapes = [], [], [], []
    partition_name = (nc.partition_id_tensor.name
                      if nc.partition_id_tensor else None)
    for alloc in nc.m.functions[0].allocations:
        if not isinstance(alloc, mybir.MemoryLocationSet):
            continue
        name = alloc.memorylocations[0].name
        if alloc.kind == "ExternalInput":
            if name != partition_name:
                in_names.append(name)
        elif alloc.kind == "ExternalOutput":
            out_names.append(name)
            shape = tuple(alloc.tensor_shape)
            dtype = mybir.dt.np(alloc.dtype)
            out_avals.append(jax.core.ShapedArray(shape, dtype))
            zero_shapes.append((shape, dtype))
    n_params = len(in_names)
    n_outs = len(out_avals)
    all_in_names = list(in_names) + list(out_names)
    if partition_name is not None:
        all_in_names.append(partition_name)
    donate = tuple(range(n_params, n_params + n_outs))

    def _body(*args):
        operands = list(args)
        if partition_name is not None:
            operands.append(bass2jax.partition_id_tensor())
        outs = bass2jax._bass_exec_p.bind(
            *operands,
            out_avals=tuple(out_avals),
            in_names=tuple(all_in_names),
            out_names=tuple(out_names),
            lowering_input_output_aliases=(),
            sim_require_finite=True,
            sim_require_nnan=True,
            nc=nc,
        )
        return tuple(outs)

    devices = jax.devices()[:NCORES]
    mesh = Mesh(np.asarray(devices), ("core",))
    specs = [PartitionSpec() if name == "table_t" else PartitionSpec("core")
             for name in in_names]
    in_specs = tuple(specs) + (PartitionSpec("core"),) * n_outs
    out_specs = (PartitionSpec("core"),) * len(out_names)
    fn = jax.jit(
        shard_map(_body, mesh=mesh, in_specs=in_specs, out_specs=out_specs,
                  check_rep=False),
        donate_argnums=donate, keep_unused=True,
    )
    return fn, mesh, in_names, out_names, zero_shapes


def _percore_sorted(keys, mask, c):
    """Per-slot valid-first key ordering for core c."""
    k = np.asarray(keys[c * BL:(c + 1) * BL]).reshape(SL, N)
    m = np.asarray(mask[c * BL:(c + 1) * BL]).reshape(SL, N) != 0
    order = np.argsort(~m, axis=1, kind="stable")
    ksort = np.take_along_axis(k, order, axis=1).astype(np.int64)
    vcnt = m.sum(axis=1)
    return ksort, vcnt, m


def needed_ocols(keys, mask):
    """Per-occurrence overflow column counts (max over cores)."""
    mx = [0] * (N - C)
    for c in range(NCORES):
        _, vcnt, _ = _percore_sorted(keys, mask, c)
        for j in range(N - C):
            cnt = int((vcnt > C + j).sum())
            mx[j] = max(mx[j], (cnt + P - 1) // P)
    while mx and mx[-1] == 0:
        mx.pop()
    return tuple(mx)


def marshal_inputs(keys, mask, ocols):
    OC = sum(ocols)
    ocw = max(OC, 1)
    skeys_g = np.empty((NCORES * P, NT * C), np.int32)
    okeys_g = np.full((NCORES * P, ocw), V, np.int32)
    osid_g = np.full((NCORES * P, ocw * 8), DUMP_SLOT, np.int16)
    mask_g = np.empty((NCORES * P, NT * N), np.int32)
    ooff = np.concatenate(([0], np.cumsum(ocols))).astype(int)
    for c in range(NCORES):
        ksort, vcnt, m = _percore_sorted(keys, mask, c)
        static = ksort[:, :C].copy()
        static[np.arange(C)[None, :] >= vcnt[:, None]] = V
        skeys_g[c * P:(c + 1) * P] = (
            static.reshape(NT, P, C).transpose(1, 0, 2)
            .reshape(P, NT * C).astype(np.int32))
        mask_g[c * P:(c + 1) * P] = (
            m.reshape(NT, P, N).transpose(1, 0, 2)
            .reshape(P, NT * N).astype(np.int32))
        for j in range(len(ocols)):
            sel = np.flatnonzero(vcnt > C + j)       # slots with occurrence j
            kj = ksort[sel, C + j]
            cj = ocols[j]
            if len(sel) > cj * P:
                raise OverflowError(f"occurrence {j}: {len(sel)} > {cj * P}")
            kflat = np.full(cj * P, V, np.int64)
            sflat = np.full(cj * P, DUMP_SLOT, np.int64)
            kflat[:len(sel)] = kj
            sflat[:len(sel)] = sel
            # source cell for stream pos i: partition i%128, col off + i//128
            okeys_g[c * P:(c + 1) * P, ooff[j]:ooff[j] + cj] = (
                kflat.reshape(cj, P).T.astype(np.int32))
            # idx wrapped layout per call: pos i -> [i%16, i//16], replicated
            w = sflat.reshape(cj * 8, 16).T.astype(np.int16)   # [16, cj*8]
            osid_g[c * P:(c + 1) * P, ooff[j] * 8:(ooff[j] + cj) * 8] = (
                np.tile(w, (8, 1)))
    return {"skeys_t": skeys_g, "okeys_t": okeys_g, "osid_t": osid_g,
            "mask_t": mask_g}


def pad_table(table):
    t = np.zeros((VPAD, E), np.float32)
    t[:V] = table
    return t


def unmarshal_output(out_g):
    out = np.empty((B, S, E), np.float32)
    for c in range(NCORES):
        oc = np.asarray(out_g[c * P:(c + 1) * P])
        out[c * BL:(c + 1) * BL] = (
            oc.reshape(P, NT, E).transpose(1, 0, 2).reshape(BL, S, E))
    return out


def _get_state(ocols):
    if _STATE.get("ocols") != ocols:
        nc = _build_nc(ocols)
        fn, mesh, in_names, out_names, zero_shapes = _make_runner(nc)
        _STATE.update(ocols=ocols, nc=nc, fn=fn, mesh=mesh,
                      in_names=in_names, out_names=out_names,
                      zero_shapes=zero_shapes, table_key=None)
    return _STATE


def kernel(keys, mask, table):
    import jax
    from jax.sharding import NamedSharding, PartitionSpec

    ocols = needed_ocols(keys, mask)
    st = _get_state(ocols)
    ins = marshal_inputs(keys, mask, ocols)

    tkey = id(table)
    if st.get("table_key") != tkey:
        st["table_dev"] = jax.device_put(
            pad_table(np.asarray(table, dtype=np.float32)),
            NamedSharding(st["mesh"], PartitionSpec()))
        st["table_key"] = tkey
    ins["table_t"] = st["table_dev"]

    args = [ins[name] for name in st["in_names"]]
    zshape, zdtype = st["zero_shapes"][0]
    zeros_out = np.zeros((NCORES * zshape[0], *zshape[1:]), zdtype)
    outs = st["fn"](*args, zeros_out)
    out_g = np.asarray(jax.block_until_ready(outs[0]))
    return unmarshal_output(out_g)
